# revision 20
# baseline (speedup 1.0000x reference)
"""2-layer GCN encoder (PyG GCNConv style) on 8 Trainium2 NeuronCores.

Strategy (node partitioning per the sharding hint):
- Nodes are partitioned into 8 contiguous shards (6250 per core); each core
  owns the aggregation for its shard's target nodes.
- Edges (with self-loops) are sorted by target and bucketed per core /
  per 128-target block; within a block they are split into two source
  "halves" (dma_gather indices are int16, so message tables are addressed
  as two <32768-row halves) and sorted by source for HBM locality.
- Per core: h1 = (D^-1/2 x) @ W1 is computed redundantly on all cores
  (a full-x GEMM is cheaper than an AllGather of h1); per-edge messages are
  fetched with SWDGE dma_gather (round-robined over all 4 SWDGE queues =
  all 4 Q7 core pairs, since descriptor generation is the bottleneck);
  the scatter-add is a PE matmul against an on-the-fly one-hot selector
  built on DVE (S[e, t] = (col_local[e] == t)); PSUM accumulates one
  128-target block per half-pass into an SBUF f32 accumulator; the epilogue
  applies the target-side scale + ReLU on ACT.
- relu(out1)*D^-1/2 shards are AllGathered in two pieces (the first fires
  mid-aggregation), then layer 2 repeats the structure with W2, reading
  transposed panels of the gathered activations. The half-split of every
  message table matches the producing GEMM's write order, so each half of
  the next phase's gathers can start as soon as its half-table is ready.

The program is specialized to the input graph at run time: the edge
schedule (chunks per block) is compiled into the instruction stream, kept
uniform across cores (max over cores per block) so one SPMD program serves
all 8 cores.
"""

import glob
import sys

_b16 = sorted(glob.glob("/nix/store/*-b16-bazel-*/lib/python3.13/site-packages"))
if _b16 and _b16[-1] not in sys.path:
    sys.path.insert(0, _b16[-1])
if "/opt/trn_rl_repo" not in sys.path:
    sys.path.insert(1, "/opt/trn_rl_repo")

from dataclasses import dataclass

import ml_dtypes
import numpy as np

import concourse.bacc as bacc
import concourse.mybir as mybir
import concourse.tile as tile
from concourse.bass_utils import run_bass_kernel_spmd
from concourse.library_config import mlp

BF16 = mybir.dt.bfloat16
F32 = mybir.dt.float32
I16 = mybir.dt.int16
BF = ml_dtypes.bfloat16


@dataclass
class Cfg:
    n_nodes: int = 50000
    in_ch: int = 256
    hid: int = 128
    r: int = 8              # cores
    blk: int = 128          # targets per psum block
    chunk: int = 128        # edges per matmul chunk
    gcap: int = 8           # chunks per dma_gather call (1024 idxs)
    gemm_panel: int = 4096  # node columns per lhsT panel (GEMM1)

    @property
    def npc(self):
        return self.n_nodes // self.r

    @property
    def nblk(self):
        return -(-self.npc // self.blk)

    @property
    def pad_shard(self):
        return self.nblk * self.blk

    # --- layer-1 message-table split (by absolute node id, aligned to a
    # GEMM1 panel boundary so the first half-table completes early) ---
    @property
    def split1(self):
        if self.n_nodes <= self.gemm_panel:
            return self.n_nodes // 2
        return max((self.n_nodes // 2 // self.gemm_panel) * self.gemm_panel,
                   self.gemm_panel)

    # --- layer-2 split: blocks [0, nblk_a) are AllGathered first ---
    @property
    def nblk_a(self):
        return self.nblk // 2

    @property
    def rows_a(self):  # per-rank rows in region A
        return self.nblk_a * self.blk

    @property
    def rows_b(self):
        return self.pad_shard - self.rows_a


def _wrap_idx(a):
    # logical i -> [i % 16, i // 16], replicated to 128 partitions
    a = np.asarray(a, np.int16)
    assert len(a) % 16 == 0
    return np.ascontiguousarray(np.tile(a.reshape(-1, 16).T, (8, 1)))


def _wrap_col(a):
    # chunk-major: edge j of chunk q -> [j, q]
    a = np.asarray(a, np.float32)
    assert len(a) % 128 == 0
    return np.ascontiguousarray(a.reshape(-1, 128).T.astype(BF))


def _bucket(row, col, cfg, half_of, idx_of, tag):
    """Sort edges by (core, block, half, row); build per-core padded
    streams. Returns nch [nblk, 2] and per-core dict of idx/col arrays."""
    R, NPC, BLK, NBLK, CH = cfg.r, cfg.npc, cfg.blk, cfg.nblk, cfg.chunk
    core = col // NPC
    blk = (col % NPC) // BLK
    hi = half_of(row).astype(np.int64)
    order = np.lexsort((row, hi, blk, core))
    row_s, col_s = row[order], col[order]
    core_s, blk_s, hi_s = core[order], blk[order], hi[order]

    key = (core_s * NBLK + blk_s) * 2 + hi_s
    counts = np.bincount(key, minlength=R * NBLK * 2).reshape(R, NBLK, 2)
    nch = np.maximum(-(-counts // CH), 1).max(axis=0)  # [NBLK, 2]

    seg_starts = np.zeros(R * NBLK * 2 + 1, np.int64)
    np.cumsum(counts.reshape(-1), out=seg_starts[1:])

    # a pad source row for each half (any valid source of that half)
    pad_row_val = [int(row[np.flatnonzero(hi == h)[0]])
                   if (hi == h).any() else 0 for h in (0, 1)]

    per_core = []
    for c in range(R):
        arrs = {}
        for h in (0, 1):
            rows_list, cols_list = [], []
            for b in range(NBLK):
                k = (c * NBLK + b) * 2 + h
                s, e = seg_starts[k], seg_starts[k + 1]
                pad = nch[b, h] * CH - (e - s)
                rows_list += [row_s[s:e],
                              np.full(pad, pad_row_val[h], np.int64)]
                cols_list += [col_s[s:e] - c * NPC - b * BLK,
                              np.full(pad, 255, np.int64)]
            rows = np.concatenate(rows_list)
            cols = np.concatenate(cols_list)
            idx = idx_of(rows, h)
            assert 0 <= idx.min() and idx.max() < 32768, (tag, idx.min(),
                                                          idx.max())
            arrs[f"idx{tag}{h}"] = _wrap_idx(idx)
            arrs[f"col{tag}{h}"] = _wrap_col(cols)
        per_core.append(arrs)
    return nch, per_core


def preprocess(edge_index, cfg: Cfg):
    N, R, NPC, BLK, NBLK = cfg.n_nodes, cfg.r, cfg.npc, cfg.blk, cfg.nblk
    ei = np.asarray(edge_index)
    loops = np.arange(N, dtype=np.int64)
    row = np.concatenate([ei[0].astype(np.int64), loops])
    col = np.concatenate([ei[1].astype(np.int64), loops])

    deg = np.bincount(col, minlength=N).astype(np.float64)
    dinv = np.where(deg > 0, 1.0 / np.sqrt(deg), 0.0).astype(np.float32)

    # layer 1: table = h1 in node order, halves split at split1
    nch1, pc1 = _bucket(
        row, col, cfg,
        half_of=lambda rows: rows >= cfg.split1,
        idx_of=lambda rows, h: rows - h * cfg.split1,
        tag="1")

    # layer 2: table = h2 in (region, rank, local) order
    def idx2(rows, h):
        rank, local = rows // NPC, rows % NPC
        if h == 0:
            return rank * cfg.rows_a + local
        return rank * cfg.rows_b + (local - cfg.rows_a)

    nch2, pc2 = _bucket(
        row, col, cfg,
        half_of=lambda rows: (rows % NPC) >= cfg.rows_a,
        idx_of=idx2,
        tag="2")

    per_core = []
    for c in range(R):
        arrs = {}
        arrs.update(pc1[c])
        arrs.update(pc2[c])
        dt = np.zeros((128, NBLK), np.float32)
        for b in range(NBLK):
            lo = c * NPC + b * BLK
            n = min(BLK, NPC - b * BLK)
            dt[:n, b] = dinv[lo:lo + n]
        arrs["dinv_t"] = dt
        arrs["dinv_tsq"] = dt * dt
        per_core.append(arrs)
    return (nch1, nch2), per_core, dinv


def build_program(cfg: Cfg, nchs, has_b1: bool, has_b2: bool):
    N, R, HID = cfg.n_nodes, cfg.r, cfg.hid
    NBLK, BLK, CH = cfg.nblk, cfg.blk, cfg.chunk
    nch1, nch2 = nchs
    T = {}
    loff = {}
    for l, nch in ((1, nch1), (2, nch2)):
        for h in (0, 1):
            T[(l, h)] = int(nch[:, h].sum())
        lf = np.zeros((NBLK, 2), np.int64)
        lf[1:, 0] = np.cumsum(nch[:-1, 0])
        lf[1:, 1] = np.cumsum(nch[:-1, 1])
        loff[l] = lf

    nc = bacc.Bacc("TRN2", num_devices=R, num_swdge_queues=4)

    xT = nc.dram_tensor("xT", [cfg.in_ch, N], BF16, kind="ExternalInput")
    w1 = nc.dram_tensor("W1", [cfg.in_ch, HID], BF16, kind="ExternalInput")
    w2 = nc.dram_tensor("W2", [HID, HID], BF16, kind="ExternalInput")
    iota_in = nc.dram_tensor("iota", [128, 128], BF16, kind="ExternalInput")
    dinv_t_in = nc.dram_tensor("dinv_t", [128, NBLK], F32,
                               kind="ExternalInput")
    dinv_tsq_in = nc.dram_tensor("dinv_tsq", [128, NBLK], F32,
                                 kind="ExternalInput")
    idx_ins = {(l, h): nc.dram_tensor(f"idx{l}{h}", [128, T[(l, h)] * 8],
                                      I16, kind="ExternalInput")
               for l in (1, 2) for h in (0, 1)}
    col_ins = {(l, h): nc.dram_tensor(f"col{l}{h}", [128, T[(l, h)]], BF16,
                                      kind="ExternalInput")
               for l in (1, 2) for h in (0, 1)}
    b_ins = {}
    if has_b1:
        b_ins[1] = nc.dram_tensor("b1b", [128, HID], F32,
                                  kind="ExternalInput")
    if has_b2:
        b_ins[2] = nc.dram_tensor("b2b", [128, HID], F32,
                                  kind="ExternalInput")
    out = nc.dram_tensor("out", [cfg.npc, HID], F32, kind="ExternalOutput")

    # message tables: one DRAM tensor per half so gathers depend only on
    # the half they actually read
    h1t = [nc.dram_tensor("h1lo", [cfg.split1, HID], BF16),
           nc.dram_tensor("h1hi", [N - cfg.split1, HID], BF16)]
    h2t = [nc.dram_tensor("h2a", [R * cfg.rows_a, HID], BF16),
           nc.dram_tensor("h2b", [R * cfg.rows_b, HID], BF16)]
    r1s = [nc.dram_tensor("r1sa", [cfg.rows_a, HID], BF16),
           nc.dram_tensor("r1sb", [cfg.rows_b, HID], BF16)]
    r1f = [nc.dram_tensor("r1fa", [R * cfg.rows_a, HID], BF16,
                          addr_space="Shared"),
           nc.dram_tensor("r1fb", [R * cfg.rows_b, HID], BF16,
                          addr_space="Shared")]

    with tile.TileContext(nc) as tc:
        with (
            tc.tile_pool(name="const", bufs=1) as cpool,
            tc.tile_pool(name="idx", bufs=1) as ipool,
            tc.tile_pool(name="acc", bufs=1) as apool,
            tc.tile_pool(name="panel", bufs=2) as panpool,
            tc.tile_pool(name="gout", bufs=3) as gopool,
            tc.tile_pool(name="gather", bufs=3) as gapool,
            tc.tile_pool(name="stile", bufs=3) as spool,
            tc.tile_pool(name="epi", bufs=3) as epool,
            tc.tile_pool(name="psum", bufs=4, space="PSUM") as ppool,
        ):
            nc.gpsimd.load_library(mlp)

            iota_t = cpool.tile([128, 128], BF16)
            nc.sync.dma_start(iota_t[:], iota_in[:])
            dinv_t_t = cpool.tile([128, NBLK], F32)
            nc.sync.dma_start(dinv_t_t[:], dinv_t_in[:])
            dinv_tsq_t = cpool.tile([128, NBLK], F32)
            nc.sync.dma_start(dinv_tsq_t[:], dinv_tsq_in[:])
            w1_t = cpool.tile([128, 2, HID], BF16)
            nc.sync.dma_start(w1_t[:, 0, :], w1[0:128, :])
            nc.sync.dma_start(w1_t[:, 1, :], w1[128:256, :])
            w2_t = cpool.tile([128, HID], BF16)
            nc.sync.dma_start(w2_t[:], w2[:])
            col_t = {}
            for (l, h), ci in col_ins.items():
                t = cpool.tile([128, T[(l, h)]], BF16, tag=f"colt{l}{h}")
                nc.sync.dma_start(t[:], ci[:])
                col_t[(l, h)] = t
            b_t = {}
            for l, bi in b_ins.items():
                b_t[l] = cpool.tile([128, HID], F32, tag=f"bt{l}")
                nc.sync.dma_start(b_t[l][:], bi[:])

            def load_idx(layer):
                tiles = []
                for h in (0, 1):
                    t = ipool.tile([128, T[(layer, h)] * 8], I16,
                                   tag=f"it{layer}{h}")
                    nc.sync.dma_start(t[:], idx_ins[(layer, h)][:])
                    tiles.append(t)
                return tiles

            # persistent f32 block accumulators (~3.2 MB), one tile per
            # block so downstream deps stay per-block
            acc_t = [apool.tile([128, HID], F32, name=f"accb{b}",
                                 tag=f"acc{b}")
                     for b in range(NBLK)]

            def gemm(layer):
                """h tables = panel.T @ W, batched PSUM-bank epilogues.
                Half-table 0 spans first so its gathers unblock early."""
                PANEL = cfg.gemm_panel if layer == 1 else 2048
                GRP = 8   # chunks per output DMA
                PSG = 4   # chunks per psum bank
                spans = []
                if layer == 1:
                    for h, tbl in enumerate(h1t):
                        rows = tbl.shape[0]
                        base = h * cfg.split1
                        for p0 in range(0, rows, PANEL):
                            spans.append((tbl, p0, base + p0,
                                          min(PANEL, rows - p0), None))
                else:
                    for h, tbl in enumerate(h2t):
                        rr = cfg.rows_a if h == 0 else cfg.rows_b
                        for r in range(R):
                            for p0 in range(0, rr, PANEL):
                                pn = min(PANEL, rr - p0)
                                spans.append((tbl, r * rr + p0,
                                              r * rr + p0, pn, h))
                for dst_dram, dbase, sbase, pn, src_h in spans:
                    if layer == 1:
                        pan = panpool.tile([128, 2, pn], BF16, tag="pan1")
                        nc.sync.dma_start(pan[:, 0, :],
                                          xT[0:128, sbase:sbase + pn])
                        nc.sync.dma_start(pan[:, 1, :],
                                          xT[128:256, sbase:sbase + pn])
                    else:
                        pan = panpool.tile([128, pn], BF16, tag="pan2")
                        nc.sync.dma_start(pan[:],
                                          r1f[src_h][sbase:sbase + pn, :],
                                          transpose=True)
                    nchunks = -(-pn // 128)
                    for g0 in range(0, nchunks, GRP):
                        gn = min(GRP, nchunks - g0)
                        osb = gopool.tile([128, GRP, HID], BF16, tag="osb")
                        for q0 in range(g0, g0 + gn, PSG):
                            qn = min(PSG, g0 + gn - q0)
                            ps = ppool.tile([128, PSG * 128], F32, tag="gps")
                            full = (pn - q0 * 128) >= qn * 128
                            for j in range(q0, q0 + qn):
                                rn = min(128, pn - j * 128)
                                w = (j - q0) * 128
                                if layer == 1:
                                    nc.tensor.matmul(
                                        ps[:rn, w:w + 128],
                                        lhsT=pan[:, 0, j * 128:j * 128 + rn],
                                        rhs=w1_t[:, 0, :],
                                        start=True, stop=False)
                                    nc.tensor.matmul(
                                        ps[:rn, w:w + 128],
                                        lhsT=pan[:, 1, j * 128:j * 128 + rn],
                                        rhs=w1_t[:, 1, :],
                                        start=False, stop=True)
                                else:
                                    nc.tensor.matmul(
                                        ps[:rn, w:w + 128],
                                        lhsT=pan[:, j * 128:j * 128 + rn],
                                        rhs=w2_t[:], start=True, stop=True)
                            if full:
                                nc.scalar.activation(
                                    osb[:, q0 - g0:q0 - g0 + qn, :],
                                    ps[:, :qn * 128]
                                    .rearrange("p (j f) -> p j f", f=HID),
                                    mybir.ActivationFunctionType.Copy)
                            else:
                                for j in range(q0, q0 + qn):
                                    rn = min(128, pn - j * 128)
                                    w = (j - q0) * 128
                                    nc.scalar.activation(
                                        osb[:rn, j - g0, :],
                                        ps[:rn, w:w + 128],
                                        mybir.ActivationFunctionType.Copy)
                        rows = min(gn * 128, pn - g0 * 128)
                        base = dbase + g0 * 128
                        nj = rows // 128
                        if nj:
                            nc.sync.dma_start(
                                dst_dram[base:base + nj * 128, :]
                                .rearrange("(j p) f -> p j f", p=128),
                                osb[:, 0:nj, :])
                        rem = rows - nj * 128
                        if rem:
                            nc.sync.dma_start(
                                dst_dram[base + nj * 128:base + rows, :],
                                osb[:rem, nj, :])

            qrr = [0]

            def agg_half(layer, h, srcs, idx_tiles, nch, first,
                         post=None):
                """One half-pass over all blocks: gather + S + matmul,
                accumulated into acc_t[b]; `post(b)` emits the block
                epilogue right after the second pass's accumulate."""
                lf = loff[layer]
                for b in range(NBLK):
                    n = int(nch[b, h])
                    off = int(lf[b, h])
                    ps = ppool.tile([128, 128], F32, tag="aps")
                    dst = gapool.tile([128, n, HID], BF16, tag=f"gd{h}")
                    for s0 in range(0, n, cfg.gcap):
                        sn = min(cfg.gcap, n - s0)
                        nc.gpsimd.dma_gather(
                            dst[:, s0:s0 + sn, :], srcs[h][:],
                            idx_tiles[h][:, (off + s0) * 8:
                                         (off + s0 + sn) * 8],
                            sn * CH, sn * CH, HID,
                            queue_num=qrr[0] % 4)
                        qrr[0] += 1
                    S = spool.tile([128, n, 128], BF16, tag=f"st{h}")
                    nc.vector.tensor_tensor(
                        out=S[:],
                        in0=col_t[(layer, h)][:, off:off + n].unsqueeze(2)
                            .to_broadcast([128, n, 128]),
                        in1=iota_t[:].unsqueeze(1)
                            .to_broadcast([128, n, 128]),
                        op=mybir.AluOpType.is_equal)
                    for q in range(n):
                        nc.tensor.matmul(ps[:], lhsT=S[:, q, :],
                                         rhs=dst[:, q, :],
                                         start=(q == 0), stop=(q == n - 1))
                    if first:
                        nc.vector.tensor_copy(acc_t[b][:], ps[:])
                    else:
                        nc.vector.tensor_tensor(
                            out=acc_t[b][:], in0=acc_t[b][:],
                            in1=ps[:], op=mybir.AluOpType.add)
                        if post is not None:
                            post(b)

            def write1(b):
                rsb = epool.tile([128, HID], BF16, tag="rsb")
                src_ap = acc_t[b][:]
                if not has_b1:
                    nc.scalar.activation(
                        rsb[:], src_ap, mybir.ActivationFunctionType.Relu,
                        scale=dinv_tsq_t[:, b:b + 1])
                else:
                    tmp = epool.tile([128, HID], F32, tag="tmp1")
                    nc.vector.tensor_scalar_mul(tmp[:], src_ap,
                                                dinv_t_t[:, b:b + 1])
                    nc.vector.tensor_tensor(out=tmp[:], in0=tmp[:],
                                            in1=b_t[1][:],
                                            op=mybir.AluOpType.add)
                    # dinv * relu(y) == relu(dinv * y) for dinv > 0
                    nc.scalar.activation(rsb[:], tmp[:],
                                         mybir.ActivationFunctionType.Relu,
                                         scale=dinv_t_t[:, b:b + 1])
                if b < cfg.nblk_a:
                    nc.sync.dma_start(r1s[0][b * BLK:(b + 1) * BLK, :],
                                      rsb[:])
                else:
                    bb = b - cfg.nblk_a
                    nc.sync.dma_start(r1s[1][bb * BLK:(bb + 1) * BLK, :],
                                      rsb[:])

            # ---- Phase 1: h1 = (D^-1/2 x) @ W1 (x pre-scaled on host) ----
            idx_l1 = load_idx(1)
            gemm(layer=1)

            # ---- Phase 2: layer-1 aggregation, then epilogues ----
            agg_half(1, 0, h1t, idx_l1, nch1, first=True)
            agg_half(1, 1, h1t, idx_l1, nch1, first=False, post=write1)

            # ---- Phase 3: staged AllGather (A on DVE, B on GpSimd) ----
            nc.gpsimd.collective_compute(
                "AllGather", mybir.AluOpType.bypass,
                replica_groups=[list(range(R))],
                ins=[r1s[0][:]], outs=[r1f[0][:]])
            nc.gpsimd.collective_compute(
                "AllGather", mybir.AluOpType.bypass,
                replica_groups=[list(range(R))],
                ins=[r1s[1][:]], outs=[r1f[1][:]])

            # ---- Phase 4: h2 = (D^-1/2 relu(out1)) @ W2 ----
            idx_l2 = load_idx(2)
            gemm(layer=2)

            def write2(b):
                osb2 = epool.tile([128, HID], F32, tag="osb2")
                nc.scalar.activation(
                    osb2[:], acc_t[b][:],
                    mybir.ActivationFunctionType.Copy,
                    scale=dinv_t_t[:, b:b + 1])
                if has_b2:
                    nc.vector.tensor_tensor(out=osb2[:], in0=osb2[:],
                                            in1=b_t[2][:],
                                            op=mybir.AluOpType.add)
                rows = min(BLK, cfg.npc - b * BLK)
                nc.sync.dma_start(out[b * BLK:b * BLK + rows, :],
                                  osb2[:rows, :])

            # ---- Phase 5: layer-2 aggregation -> out (f32) ----
            agg_half(2, 0, h2t, idx_l2, nch2, first=True)
            agg_half(2, 1, h2t, idx_l2, nch2, first=False, post=write2)

    nc.compile()
    return nc


def make_in_maps(cfg: Cfg, per_core, x, dinv, W1, b1, W2, b2):
    xs = (np.asarray(x, np.float32) * dinv[:, None])
    xT = np.ascontiguousarray(xs.T).astype(BF)
    w1b = np.asarray(W1, np.float32).astype(BF)
    w2b = np.asarray(W2, np.float32).astype(BF)
    iota = np.tile(np.arange(128, dtype=np.float32), (128, 1)).astype(BF)
    has_b1 = bool(np.any(np.asarray(b1)))
    has_b2 = bool(np.any(np.asarray(b2)))
    in_maps = []
    for c in range(cfg.r):
        m = {"xT": xT, "W1": w1b, "W2": w2b, "iota": iota}
        m.update(per_core[c])
        if has_b1:
            m["b1b"] = np.tile(np.asarray(b1, np.float32), (128, 1))
        if has_b2:
            m["b2b"] = np.tile(np.asarray(b2, np.float32), (128, 1))
        in_maps.append(m)
    return in_maps, has_b1, has_b2


def kernel(x, edge_index, W1, b1, W2, b2):
    cfg = Cfg()
    nchs, per_core, dinv = preprocess(edge_index, cfg)
    in_maps, has_b1, has_b2 = make_in_maps(cfg, per_core, x, dinv,
                                           W1, b1, W2, b2)
    nc = build_program(cfg, nchs, has_b1, has_b2)
    res = run_bass_kernel_spmd(nc, in_maps, list(range(cfg.r)))
    return np.concatenate([res.results[c]["out"] for c in range(cfg.r)],
                          axis=0)


# revision 21
# speedup vs baseline: 1.0111x; 1.0111x over previous
"""2-layer GCN encoder (PyG GCNConv style) on 8 Trainium2 NeuronCores.

Strategy (node partitioning per the sharding hint):
- Nodes are partitioned into 8 contiguous shards (6250 per core); each core
  owns the aggregation for its shard's target nodes.
- Edges (with self-loops) are sorted by target and bucketed per core /
  per 128-target block; within a block they are split into two source
  "halves" (dma_gather indices are int16, so message tables are addressed
  as two <32768-row halves) and sorted by source for HBM locality.
- Per core: h1 = (D^-1/2 x) @ W1 is computed redundantly on all cores
  (a full-x GEMM is cheaper than an AllGather of h1); per-edge messages are
  fetched with SWDGE dma_gather (round-robined over all 4 SWDGE queues =
  all 4 Q7 core pairs, since descriptor generation is the bottleneck);
  the scatter-add is a PE matmul against an on-the-fly one-hot selector
  built on DVE (S[e, t] = (col_local[e] == t)); PSUM accumulates one
  128-target block per half-pass into an SBUF f32 accumulator; the epilogue
  applies the target-side scale + ReLU on ACT.
- relu(out1)*D^-1/2 shards are AllGathered in two pieces (the first fires
  mid-aggregation), then layer 2 repeats the structure with W2, reading
  transposed panels of the gathered activations. The half-split of every
  message table matches the producing GEMM's write order, so each half of
  the next phase's gathers can start as soon as its half-table is ready.

The program is specialized to the input graph at run time: the edge
schedule (chunks per block) is compiled into the instruction stream, kept
uniform across cores (max over cores per block) so one SPMD program serves
all 8 cores.
"""

import glob
import sys

_b16 = sorted(glob.glob("/nix/store/*-b16-bazel-*/lib/python3.13/site-packages"))
if _b16 and _b16[-1] not in sys.path:
    sys.path.insert(0, _b16[-1])
if "/opt/trn_rl_repo" not in sys.path:
    sys.path.insert(1, "/opt/trn_rl_repo")

from dataclasses import dataclass

import ml_dtypes
import numpy as np

import concourse.bacc as bacc
import concourse.mybir as mybir
import concourse.tile as tile
from concourse.bass_utils import run_bass_kernel_spmd
from concourse.library_config import mlp

BF16 = mybir.dt.bfloat16
F32 = mybir.dt.float32
I16 = mybir.dt.int16
BF = ml_dtypes.bfloat16


@dataclass
class Cfg:
    n_nodes: int = 50000
    in_ch: int = 256
    hid: int = 128
    r: int = 8              # cores
    blk: int = 128          # targets per psum block
    chunk: int = 128        # edges per matmul chunk
    gcap: int = 8           # chunks per dma_gather call (1024 idxs)
    gemm_panel: int = 4096  # node columns per lhsT panel (GEMM1)

    @property
    def npc(self):
        return self.n_nodes // self.r

    @property
    def nblk(self):
        return -(-self.npc // self.blk)

    @property
    def pad_shard(self):
        return self.nblk * self.blk

    # --- layer-1 message-table split (by absolute node id, aligned to a
    # GEMM1 panel boundary so the first half-table completes early) ---
    @property
    def split1(self):
        if self.n_nodes <= self.gemm_panel:
            return self.n_nodes // 2
        return max((self.n_nodes // 2 // self.gemm_panel) * self.gemm_panel,
                   self.gemm_panel)

    # --- layer-2 split: blocks [0, nblk_a) are AllGathered first ---
    @property
    def nblk_a(self):
        return self.nblk // 2

    @property
    def rows_a(self):  # per-rank rows in region A
        return self.nblk_a * self.blk

    @property
    def rows_b(self):
        return self.pad_shard - self.rows_a


def _wrap_idx(a):
    # logical i -> [i % 16, i // 16], replicated to 128 partitions
    a = np.asarray(a, np.int16)
    assert len(a) % 16 == 0
    return np.ascontiguousarray(np.tile(a.reshape(-1, 16).T, (8, 1)))


def _wrap_col(a):
    # chunk-major: edge j of chunk q -> [j, q]
    a = np.asarray(a, np.float32)
    assert len(a) % 128 == 0
    return np.ascontiguousarray(a.reshape(-1, 128).T.astype(BF))


def _bucket(row, col, cfg, half_of, idx_of, tag):
    """Sort edges by (core, block, half, row); build per-core padded
    streams. Returns nch [nblk, 2] and per-core dict of idx/col arrays."""
    R, NPC, BLK, NBLK, CH = cfg.r, cfg.npc, cfg.blk, cfg.nblk, cfg.chunk
    core = col // NPC
    blk = (col % NPC) // BLK
    hi = half_of(row).astype(np.int64)
    order = np.lexsort((row, hi, blk, core))
    row_s, col_s = row[order], col[order]
    core_s, blk_s, hi_s = core[order], blk[order], hi[order]

    key = (core_s * NBLK + blk_s) * 2 + hi_s
    counts = np.bincount(key, minlength=R * NBLK * 2).reshape(R, NBLK, 2)
    nch = np.maximum(-(-counts // CH), 1).max(axis=0)  # [NBLK, 2]

    seg_starts = np.zeros(R * NBLK * 2 + 1, np.int64)
    np.cumsum(counts.reshape(-1), out=seg_starts[1:])

    # a pad source row for each half (any valid source of that half)
    pad_row_val = [int(row[np.flatnonzero(hi == h)[0]])
                   if (hi == h).any() else 0 for h in (0, 1)]

    per_core = []
    for c in range(R):
        arrs = {}
        for h in (0, 1):
            rows_list, cols_list = [], []
            for b in range(NBLK):
                k = (c * NBLK + b) * 2 + h
                s, e = seg_starts[k], seg_starts[k + 1]
                pad = nch[b, h] * CH - (e - s)
                rows_list += [row_s[s:e],
                              np.full(pad, pad_row_val[h], np.int64)]
                cols_list += [col_s[s:e] - c * NPC - b * BLK,
                              np.full(pad, 255, np.int64)]
            rows = np.concatenate(rows_list)
            cols = np.concatenate(cols_list)
            idx = idx_of(rows, h)
            assert 0 <= idx.min() and idx.max() < 32768, (tag, idx.min(),
                                                          idx.max())
            arrs[f"idx{tag}{h}"] = _wrap_idx(idx)
            arrs[f"col{tag}{h}"] = _wrap_col(cols)
        per_core.append(arrs)
    return nch, per_core


def preprocess(edge_index, cfg: Cfg):
    N, R, NPC, BLK, NBLK = cfg.n_nodes, cfg.r, cfg.npc, cfg.blk, cfg.nblk
    ei = np.asarray(edge_index)
    loops = np.arange(N, dtype=np.int64)
    row = np.concatenate([ei[0].astype(np.int64), loops])
    col = np.concatenate([ei[1].astype(np.int64), loops])

    deg = np.bincount(col, minlength=N).astype(np.float64)
    dinv = np.where(deg > 0, 1.0 / np.sqrt(deg), 0.0).astype(np.float32)

    # layer 1: table = h1 in node order, halves split at split1
    nch1, pc1 = _bucket(
        row, col, cfg,
        half_of=lambda rows: rows >= cfg.split1,
        idx_of=lambda rows, h: rows - h * cfg.split1,
        tag="1")

    # layer 2: table = h2 in (region, rank, local) order
    def idx2(rows, h):
        rank, local = rows // NPC, rows % NPC
        if h == 0:
            return rank * cfg.rows_a + local
        return rank * cfg.rows_b + (local - cfg.rows_a)

    nch2, pc2 = _bucket(
        row, col, cfg,
        half_of=lambda rows: (rows % NPC) >= cfg.rows_a,
        idx_of=idx2,
        tag="2")

    per_core = []
    for c in range(R):
        arrs = {}
        arrs.update(pc1[c])
        arrs.update(pc2[c])
        dt = np.zeros((128, NBLK), np.float32)
        for b in range(NBLK):
            lo = c * NPC + b * BLK
            n = min(BLK, NPC - b * BLK)
            dt[:n, b] = dinv[lo:lo + n]
        arrs["dinv_t"] = dt
        arrs["dinv_tsq"] = dt * dt
        per_core.append(arrs)
    return (nch1, nch2), per_core, dinv


def build_program(cfg: Cfg, nchs, has_b1: bool, has_b2: bool):
    N, R, HID = cfg.n_nodes, cfg.r, cfg.hid
    NBLK, BLK, CH = cfg.nblk, cfg.blk, cfg.chunk
    nch1, nch2 = nchs
    T = {}
    loff = {}
    for l, nch in ((1, nch1), (2, nch2)):
        for h in (0, 1):
            T[(l, h)] = int(nch[:, h].sum())
        lf = np.zeros((NBLK, 2), np.int64)
        lf[1:, 0] = np.cumsum(nch[:-1, 0])
        lf[1:, 1] = np.cumsum(nch[:-1, 1])
        loff[l] = lf

    nc = bacc.Bacc("TRN2", num_devices=R, num_swdge_queues=4)

    xT = nc.dram_tensor("xT", [cfg.in_ch, N], BF16, kind="ExternalInput")
    w1 = nc.dram_tensor("W1", [cfg.in_ch, HID], BF16, kind="ExternalInput")
    w2 = nc.dram_tensor("W2", [HID, HID], BF16, kind="ExternalInput")
    iota_in = nc.dram_tensor("iota", [128, 128], BF16, kind="ExternalInput")
    dinv_t_in = nc.dram_tensor("dinv_t", [128, NBLK], F32,
                               kind="ExternalInput")
    dinv_tsq_in = nc.dram_tensor("dinv_tsq", [128, NBLK], F32,
                                 kind="ExternalInput")
    idx_ins = {(l, h): nc.dram_tensor(f"idx{l}{h}", [128, T[(l, h)] * 8],
                                      I16, kind="ExternalInput")
               for l in (1, 2) for h in (0, 1)}
    col_ins = {(l, h): nc.dram_tensor(f"col{l}{h}", [128, T[(l, h)]], BF16,
                                      kind="ExternalInput")
               for l in (1, 2) for h in (0, 1)}
    b_ins = {}
    if has_b1:
        b_ins[1] = nc.dram_tensor("b1b", [128, HID], F32,
                                  kind="ExternalInput")
    if has_b2:
        b_ins[2] = nc.dram_tensor("b2b", [128, HID], F32,
                                  kind="ExternalInput")
    out = nc.dram_tensor("out", [cfg.npc, HID], F32, kind="ExternalOutput")

    # message tables: one DRAM tensor per half so gathers depend only on
    # the half they actually read
    h1t = [nc.dram_tensor("h1lo", [cfg.split1, HID], BF16),
           nc.dram_tensor("h1hi", [N - cfg.split1, HID], BF16)]
    h2t = [nc.dram_tensor("h2a", [R * cfg.rows_a, HID], BF16),
           nc.dram_tensor("h2b", [R * cfg.rows_b, HID], BF16)]
    r1s = [nc.dram_tensor("r1sa", [cfg.rows_a, HID], BF16),
           nc.dram_tensor("r1sb", [cfg.rows_b, HID], BF16)]
    r1f = [nc.dram_tensor("r1fa", [R * cfg.rows_a, HID], BF16,
                          addr_space="Shared"),
           nc.dram_tensor("r1fb", [R * cfg.rows_b, HID], BF16,
                          addr_space="Shared")]

    with tile.TileContext(nc) as tc:
        with (
            tc.tile_pool(name="const", bufs=1) as cpool,
            tc.tile_pool(name="idx", bufs=1) as ipool,
            tc.tile_pool(name="acc", bufs=1) as apool,
            tc.tile_pool(name="panel", bufs=2) as panpool,
            tc.tile_pool(name="gout", bufs=3) as gopool,
            tc.tile_pool(name="gather", bufs=3) as gapool,
            tc.tile_pool(name="stile", bufs=3) as spool,
            tc.tile_pool(name="epi", bufs=3) as epool,
            tc.tile_pool(name="psum", bufs=4, space="PSUM") as ppool,
        ):
            nc.gpsimd.load_library(mlp)

            iota_t = cpool.tile([128, 128], BF16)
            nc.sync.dma_start(iota_t[:], iota_in[:])
            dinv_t_t = cpool.tile([128, NBLK], F32)
            nc.sync.dma_start(dinv_t_t[:], dinv_t_in[:])
            dinv_tsq_t = cpool.tile([128, NBLK], F32)
            nc.sync.dma_start(dinv_tsq_t[:], dinv_tsq_in[:])
            w1_t = cpool.tile([128, 2, HID], BF16)
            nc.sync.dma_start(w1_t[:, 0, :], w1[0:128, :])
            nc.sync.dma_start(w1_t[:, 1, :], w1[128:256, :])
            w2_t = cpool.tile([128, HID], BF16)
            nc.sync.dma_start(w2_t[:], w2[:])
            col_t = {}
            for (l, h), ci in col_ins.items():
                t = cpool.tile([128, T[(l, h)]], BF16, tag=f"colt{l}{h}")
                nc.sync.dma_start(t[:], ci[:])
                col_t[(l, h)] = t
            b_t = {}
            for l, bi in b_ins.items():
                b_t[l] = cpool.tile([128, HID], F32, tag=f"bt{l}")
                nc.sync.dma_start(b_t[l][:], bi[:])

            def load_idx(layer):
                tiles = []
                for h in (0, 1):
                    t = ipool.tile([128, T[(layer, h)] * 8], I16,
                                   tag=f"it{layer}{h}")
                    nc.sync.dma_start(t[:], idx_ins[(layer, h)][:])
                    tiles.append(t)
                return tiles

            # persistent f32 block accumulators (~3.2 MB), one tile per
            # block so downstream deps stay per-block
            acc_t = [apool.tile([128, HID], F32, name=f"accb{b}",
                                 tag=f"acc{b}")
                     for b in range(NBLK)]

            def gemm(layer):
                """h tables = panel.T @ W, batched PSUM-bank epilogues.
                Half-table 0 spans first so its gathers unblock early."""
                PANEL = cfg.gemm_panel if layer == 1 else 2048
                GRP = 8   # chunks per output DMA
                PSG = 4   # chunks per psum bank
                spans = []
                if layer == 1:
                    for h, tbl in enumerate(h1t):
                        rows = tbl.shape[0]
                        base = h * cfg.split1
                        for p0 in range(0, rows, PANEL):
                            spans.append((tbl, p0, base + p0,
                                          min(PANEL, rows - p0), None))
                else:
                    for h, tbl in enumerate(h2t):
                        rr = cfg.rows_a if h == 0 else cfg.rows_b
                        for r in range(R):
                            for p0 in range(0, rr, PANEL):
                                pn = min(PANEL, rr - p0)
                                spans.append((tbl, r * rr + p0,
                                              r * rr + p0, pn, h))
                for dst_dram, dbase, sbase, pn, src_h in spans:
                    if layer == 1:
                        pan = panpool.tile([128, 2, pn], BF16, tag="pan1")
                        nc.sync.dma_start(pan[:, 0, :],
                                          xT[0:128, sbase:sbase + pn])
                        nc.sync.dma_start(pan[:, 1, :],
                                          xT[128:256, sbase:sbase + pn])
                    else:
                        pan = panpool.tile([128, pn], BF16, tag="pan2")
                        nc.sync.dma_start(pan[:],
                                          r1f[src_h][sbase:sbase + pn, :],
                                          transpose=True)
                    nchunks = -(-pn // 128)
                    for g0 in range(0, nchunks, GRP):
                        gn = min(GRP, nchunks - g0)
                        osb = gopool.tile([128, GRP, HID], BF16, tag="osb")
                        for q0 in range(g0, g0 + gn, PSG):
                            qn = min(PSG, g0 + gn - q0)
                            ps = ppool.tile([128, PSG * 128], F32, tag="gps")
                            full = (pn - q0 * 128) >= qn * 128
                            for j in range(q0, q0 + qn):
                                rn = min(128, pn - j * 128)
                                w = (j - q0) * 128
                                if layer == 1:
                                    nc.tensor.matmul(
                                        ps[:rn, w:w + 128],
                                        lhsT=pan[:, 0, j * 128:j * 128 + rn],
                                        rhs=w1_t[:, 0, :],
                                        start=True, stop=False)
                                    nc.tensor.matmul(
                                        ps[:rn, w:w + 128],
                                        lhsT=pan[:, 1, j * 128:j * 128 + rn],
                                        rhs=w1_t[:, 1, :],
                                        start=False, stop=True)
                                else:
                                    nc.tensor.matmul(
                                        ps[:rn, w:w + 128],
                                        lhsT=pan[:, j * 128:j * 128 + rn],
                                        rhs=w2_t[:], start=True, stop=True)
                            if full:
                                nc.scalar.activation(
                                    osb[:, q0 - g0:q0 - g0 + qn, :],
                                    ps[:, :qn * 128]
                                    .rearrange("p (j f) -> p j f", f=HID),
                                    mybir.ActivationFunctionType.Copy)
                            else:
                                for j in range(q0, q0 + qn):
                                    rn = min(128, pn - j * 128)
                                    w = (j - q0) * 128
                                    nc.scalar.activation(
                                        osb[:rn, j - g0, :],
                                        ps[:rn, w:w + 128],
                                        mybir.ActivationFunctionType.Copy)
                        rows = min(gn * 128, pn - g0 * 128)
                        base = dbase + g0 * 128
                        nj = rows // 128
                        if nj:
                            nc.sync.dma_start(
                                dst_dram[base:base + nj * 128, :]
                                .rearrange("(j p) f -> p j f", p=128),
                                osb[:, 0:nj, :])
                        rem = rows - nj * 128
                        if rem:
                            nc.sync.dma_start(
                                dst_dram[base + nj * 128:base + rows, :],
                                osb[:rem, nj, :])

            qrr = [0]

            def agg_half(layer, h, srcs, idx_tiles, nch, first,
                         post=None):
                """One half-pass over all blocks: gather + S + matmul,
                accumulated into acc_t[b]; `post(b)` emits the block
                epilogue right after the second pass's accumulate."""
                lf = loff[layer]
                for b in range(NBLK):
                    n = int(nch[b, h])
                    off = int(lf[b, h])
                    ps = ppool.tile([128, 128], F32, tag="aps")
                    dst = gapool.tile([128, n, HID], BF16, tag=f"gd{h}")
                    for s0 in range(0, n, cfg.gcap):
                        sn = min(cfg.gcap, n - s0)
                        nc.gpsimd.dma_gather(
                            dst[:, s0:s0 + sn, :], srcs[h][:],
                            idx_tiles[h][:, (off + s0) * 8:
                                         (off + s0 + sn) * 8],
                            sn * CH, sn * CH, HID,
                            queue_num=qrr[0] % 4)
                        qrr[0] += 1
                    S = spool.tile([128, n, 128], BF16, tag=f"st{h}")
                    nc.vector.tensor_tensor(
                        out=S[:],
                        in0=col_t[(layer, h)][:, off:off + n].unsqueeze(2)
                            .to_broadcast([128, n, 128]),
                        in1=iota_t[:].unsqueeze(1)
                            .to_broadcast([128, n, 128]),
                        op=mybir.AluOpType.is_equal)
                    for q in range(n):
                        nc.tensor.matmul(ps[:], lhsT=S[:, q, :],
                                         rhs=dst[:, q, :],
                                         start=(q == 0), stop=(q == n - 1))
                    if first:
                        nc.vector.tensor_copy(acc_t[b][:], ps[:])
                    else:
                        nc.vector.tensor_tensor(
                            out=acc_t[b][:], in0=acc_t[b][:],
                            in1=ps[:], op=mybir.AluOpType.add)
                        if post is not None:
                            post(b)

            def write1(b):
                rsb = epool.tile([128, HID], BF16, tag="rsb")
                src_ap = acc_t[b][:]
                if not has_b1:
                    nc.scalar.activation(
                        rsb[:], src_ap, mybir.ActivationFunctionType.Relu,
                        scale=dinv_tsq_t[:, b:b + 1])
                else:
                    tmp = epool.tile([128, HID], F32, tag="tmp1")
                    nc.vector.tensor_scalar_mul(tmp[:], src_ap,
                                                dinv_t_t[:, b:b + 1])
                    nc.vector.tensor_tensor(out=tmp[:], in0=tmp[:],
                                            in1=b_t[1][:],
                                            op=mybir.AluOpType.add)
                    # dinv * relu(y) == relu(dinv * y) for dinv > 0
                    nc.scalar.activation(rsb[:], tmp[:],
                                         mybir.ActivationFunctionType.Relu,
                                         scale=dinv_t_t[:, b:b + 1])
                if b < cfg.nblk_a:
                    nc.sync.dma_start(r1s[0][b * BLK:(b + 1) * BLK, :],
                                      rsb[:])
                else:
                    bb = b - cfg.nblk_a
                    nc.sync.dma_start(r1s[1][bb * BLK:(bb + 1) * BLK, :],
                                      rsb[:])

            # ---- Phase 1: h1 = (D^-1/2 x) @ W1 (x pre-scaled on host) ----
            idx_l1 = load_idx(1)
            gemm(layer=1)

            # ---- Phase 2: layer-1 aggregation, then epilogues ----
            agg_half(1, 0, h1t, idx_l1, nch1, first=True)
            agg_half(1, 1, h1t, idx_l1, nch1, first=False, post=write1)

            # ---- Phase 3: staged AllGather (boosted so each fires the
            # moment its half-shard is written) ----
            with tc.high_priority():
                nc.gpsimd.collective_compute(
                    "AllGather", mybir.AluOpType.bypass,
                    replica_groups=[list(range(R))],
                    ins=[r1s[0][:]], outs=[r1f[0][:]])
                nc.gpsimd.collective_compute(
                    "AllGather", mybir.AluOpType.bypass,
                    replica_groups=[list(range(R))],
                    ins=[r1s[1][:]], outs=[r1f[1][:]])

            # ---- Phase 4: h2 = (D^-1/2 relu(out1)) @ W2 ----
            idx_l2 = load_idx(2)
            gemm(layer=2)

            def write2(b):
                osb2 = epool.tile([128, HID], F32, tag="osb2")
                nc.scalar.activation(
                    osb2[:], acc_t[b][:],
                    mybir.ActivationFunctionType.Copy,
                    scale=dinv_t_t[:, b:b + 1])
                if has_b2:
                    nc.vector.tensor_tensor(out=osb2[:], in0=osb2[:],
                                            in1=b_t[2][:],
                                            op=mybir.AluOpType.add)
                rows = min(BLK, cfg.npc - b * BLK)
                nc.sync.dma_start(out[b * BLK:b * BLK + rows, :],
                                  osb2[:rows, :])

            # ---- Phase 5: layer-2 aggregation -> out (f32) ----
            agg_half(2, 0, h2t, idx_l2, nch2, first=True)
            # demote pass B so its gathers never head-of-line-block the
            # Pool sequencer while h2b is still being produced
            tc.cur_priority += 500000
            agg_half(2, 1, h2t, idx_l2, nch2, first=False, post=write2)

    nc.compile()
    return nc


def make_in_maps(cfg: Cfg, per_core, x, dinv, W1, b1, W2, b2):
    xs = (np.asarray(x, np.float32) * dinv[:, None])
    xT = np.ascontiguousarray(xs.T).astype(BF)
    w1b = np.asarray(W1, np.float32).astype(BF)
    w2b = np.asarray(W2, np.float32).astype(BF)
    iota = np.tile(np.arange(128, dtype=np.float32), (128, 1)).astype(BF)
    has_b1 = bool(np.any(np.asarray(b1)))
    has_b2 = bool(np.any(np.asarray(b2)))
    in_maps = []
    for c in range(cfg.r):
        m = {"xT": xT, "W1": w1b, "W2": w2b, "iota": iota}
        m.update(per_core[c])
        if has_b1:
            m["b1b"] = np.tile(np.asarray(b1, np.float32), (128, 1))
        if has_b2:
            m["b2b"] = np.tile(np.asarray(b2, np.float32), (128, 1))
        in_maps.append(m)
    return in_maps, has_b1, has_b2


def kernel(x, edge_index, W1, b1, W2, b2):
    cfg = Cfg()
    nchs, per_core, dinv = preprocess(edge_index, cfg)
    in_maps, has_b1, has_b2 = make_in_maps(cfg, per_core, x, dinv,
                                           W1, b1, W2, b2)
    nc = build_program(cfg, nchs, has_b1, has_b2)
    res = run_bass_kernel_spmd(nc, in_maps, list(range(cfg.r)))
    return np.concatenate([res.results[c]["out"] for c in range(cfg.r)],
                          axis=0)
